# revision 6
# baseline (speedup 1.0000x reference)
"""ConvTasNet-style decoder kernel for Trainium2 (8 NeuronCores).

Computation (per batch m, channel c):
    s[n,k]    = mixture_w[n,k] * est_mask[c,n,k]          n=512, k=16000
    frames    = W @ s                                      [16, 16000]
    out[8q+r] = frames[r, q] + frames[r+8, q-1]            overlap-add, hop 8

Sharding: 8 cores = (m in 0..3) x (k-half in 0..1); each core handles both
channels of one (m, k-half); no cross-core communication. The final
overlap-add is a cheap O(T) host pass over the returned frame matrices.

The kernel is HBM-DMA-bound, so inputs ship 8-bit: mw -> int8 with a
per-row scale s[n], est_mask -> uint8 (x255). Both scales fold into the
stationary weights W'[n,l] = W[l,n] * s[n] * 2^12 / 255 (fp16; the 2^12
keeps W' out of fp16 subnormals and is divided back out on the host).
The k axis is zero-padded to KH_PAD (multiple of 128) on the host.

The mask multiply runs DIRECTLY on the 8-bit operands: the DVE and
GpSimd engines multiply u8 x i8 -> f16 at their 1-byte mixed rate, so no
mw upconversion pass exists for those bands. Work is cut into 512-column
chunks; per chunk the columns split into bands chosen by a solved
min-max ledger over engine rates + the DMA pipe:
  [0,x)  mixed u8*i8 on the DVE,
  [x,y)  ACT converts mask+mw to fp16, DVE multiplies at its 2x fp16
         rate (spends idle ACT capacity to relieve the DVE),
  [y,ck) mixed u8*i8 on GpSimd.
A few dedicated fp16 chunks (mask and pre-scaled mw ship as fp16, muls
on the DVE at the 2x rate) trade spare DMA bandwidth for vector-engine
relief; they sit whole so every mask DMA keeps a >=512B contiguous run
(sub-512B runs pay 2x in the DMA engines). The last two chunks are tiny
fp16 chunks so the post-DMA drain chain is short. Both channels of every
band go through ONE engine op via a channel-broadcast (stride-0) access
pattern on the mw operand.

The GEMM uses the product tile as the STATIONARY operand and W' as the
16-column moving operand: out[k-part, l] accumulates over the four
n-subtiles into a [128, C*KB*16] PSUM tile (one region per (channel,
k-block)). The PE row count per matmul is L=16, so the PE stays far off
the critical path at any clock p-state (on hardware the cost moves into
the weight-load path, the same total rows as the classic orientation),
and the full-128-partition PSUM layout makes the per-chunk stage copy
and output DMA 4x smaller than a 32-partition layout would be.

Scheduling: input DMAs issue on the SP queue with deep tile pools so the
DMA pipe stays saturated; each chunk's output DMA is deferred several
chunks (and alternates SP/ACT queues) so its semaphore wait is already
satisfied at issue and never head-of-line-blocks a sequencer; stage
copies run on the DVE, whose matmul dependencies are produced by its own
earlier ops.
"""

import sys

sys.path.insert(0, "/opt/trn_rl_repo")

import numpy as np

M, C, N, K, L = 4, 2, 512, 16000, 16
HOP = L // 2            # 8
KH = K // 2             # 8000 frames per core
KH_PAD = 8064           # padded to a multiple of KB=128
QH = KH + 1             # 8001 output blocks per core per channel
TH = QH * HOP           # 64008 samples per core per channel
T = (K - 1) * HOP + L   # 128008 full output samples
KB = 128                # k-block: PSUM partitions per accumulation region
NB = N // 128           # 4 contraction subtiles
SCALE_BITS = 12         # W' = W * s[n] * 2^SCALE_BITS / 255

# ---- tunables ----
# u8 chunks carry mixed/conv/pool bands; dedicated fp16 chunks (DVE-only
# muls) sit between them so every mask DMA keeps a >=512B contiguous run
# (a sub-512B run pays 2x in the DMA model). Last chunk fp16 = short tail.
import os as _os
import json as _json
_CFG = _json.loads(_os.environ.get("K2_CFG", "{}"))
CHUNKS = _CFG.get("chunks", [512, 512, 512, 512, 640, 512, 512, 512, 512,
                             512, 512, 512, 512, 512, 512, 128, 128])
F16_CHUNKS = set(_CFG.get("f16_set", [i for i, ck in enumerate(CHUNKS) if ck in (256, 128)]))
CONV_KCOLS = _CFG.get("conv", 2900)
POOL_KCOLS = _CFG.get("pool", 2400)
WARMUP_MM = 0           # PE-clock warmup matmuls during the fill
IN_BUFS = _CFG.get("in_bufs", 10)
ST_BUFS = _CFG.get("st_bufs", 6)             # product-tile depth (consumed one chunk later)
OB_BUFS = 8
PSUM_BUFS = 8
STEP = 32               # planner column quantum
OUT_DEFER = _CFG.get("out_defer", 5)
POOL_CAP = _CFG.get("pool_cap", 224)
CONV_CAP = 384
_CACHE = {}

# Engine cost model (ns/col-of-128 + fixed ns/op) from TimelineSim's
# hw_specs: DVE 0.96GHz (2x when all operands are 2-byte), ACT/Pool
# 1.2GHz, Pool multiply efficiency 0.42, copy 0.60; fixed = SBUF/PSUM
# access latency charged per op.
R_DVE_F16 = 0.5208
R_DVE_MIX = 1.0417
R_DVE_CONV = 0.5208     # u8->f16 tensor_copy (2x_2p)
R_ACT = 0.8333          # any ACT copy/conv
R_POOL_MUL = 1.9841
R_STAGE_DVE = 1.0417    # psum f32 -> sbuf f16
R_STAGE_ACT = 0.8333
F_DVE = 115.0
F_ACT = 190.0
F_POOL = 100.0


def _dma_ns(total_bytes, elem_bytes):
    """TimelineSim DMA model: descriptors/16 * max(elem*mult/22.5, 7)."""
    if total_bytes == 0:
        return 0.0
    mult = 2.0 if elem_bytes < 512 else 1.0
    ns_per_desc = max(elem_bytes * mult / 22.5, 7.0)
    return (total_bytes / elem_bytes) / 16.0 * ns_per_desc


def _chunk_dma(ck, f):
    if f == ck:
        d = _dma_ns(512 * ck * 2, 2 * ck)           # mw fp16 (pre-scaled)
    else:
        d = _dma_ns(512 * ck, ck)                   # mw int8
    if f:
        d += _dma_ns(1024 * f * 2, 2 * f)           # mask fp16 part
    if ck - f:
        d += _dma_ns(1024 * (ck - f), ck - f)       # mask u8 part
    d += _dma_ns(2 * C * L * ck, C * L * ck // 32)  # frames out (paired)
    return d


def _plan(chunks):
    """Per-chunk (f, x, y, w, stage_eng) from solved global quotas.

    Global balance (LP over engine rates + DMA pipe, T ~ 40us): fp16 ~
    1280 k-cols (whole chunks), conv ~ 1440, pool ~ 2400, rest mixed;
    mw-conv all on ACT (w=0). The first two chunks skip conv so the
    GpSimd engine (the saturated pacer) starts as soon as the first u8
    mask lands; the final u8 chunk skips pool for a short tail.
    """
    assert sum(chunks) == KH_PAD, sum(chunks)
    n = len(chunks)
    u8s = [i for i in range(n) if i not in F16_CHUNKS]
    fq = {i: (chunks[i] if i in F16_CHUNKS else 0) for i in range(n)}
    cq = {i: 0 for i in range(n)}
    pq = {i: 0 for i in range(n)}
    # pool: lighter starter on chunk 0, uniform elsewhere (the tail u8
    # chunks keep pool bands so GpSimd absorbs tail work in parallel)
    if "pool_list" in _CFG:
        for i, v in zip(u8s, _CFG["pool_list"]):
            pq[i] = v
    else:
        pq[u8s[0]] = 128
        rem = POOL_KCOLS - 128
        for i in u8s[1:]:
            pq[i] = min(POOL_CAP, int(round(rem / (len(u8s) - 1) / 32)) * 32)
    if "conv_list" in _CFG:
        for i, v in zip(u8s, _CFG["conv_list"]):
            cq[i] = v
    else:
        for i in u8s[2:]:
            cq[i] = int(round(CONV_KCOLS / (len(u8s) - 2) / 32)) * 32
    loads = {"DVE": 0.0, "ACT": 0.0, "Pool": 0.0, "DMA": 1900.0}
    plans = []
    for ci, ck in enumerate(chunks):
        f = min(fq[ci], ck)
        pool = min(pq[ci], ck - f)
        conv = min(cq[ci], ck - f - pool)
        x = ck - pool - conv          # mixed band end
        y = ck - pool                 # conv band end
        # mw conv: fp16 chunks DMA a pre-scaled fp16 mw (no conv); ACT
        # converts everywhere else
        w = 0
        loads["DMA"] += _chunk_dma(ck, f)
        loads["DVE"] += C * NB * (R_DVE_F16 * (f + y - x) + R_DVE_MIX * (x - f))
        loads["DVE"] += F_DVE * ((f > 0) + (x > f) + (y > x))
        if y > x:
            loads["ACT"] += R_ACT * C * NB * (y - x) + F_ACT
        if y < ck:
            loads["Pool"] += R_POOL_MUL * C * NB * (ck - y) + F_POOL
        if y > x:
            # conv band also needs its mw columns upconverted on ACT
            loads["ACT"] += R_ACT * NB * (y - x) + F_ACT
        scols = C * L * ck // KB
        if not _CFG.get("stage_dve", True) and loads["ACT"] + R_STAGE_ACT * scols \
                + F_ACT < loads["DVE"] + (R_STAGE_DVE * scols + F_DVE):
            loads["ACT"] += R_STAGE_ACT * scols + F_ACT
            stage_eng = "ACT"
        else:
            loads["DVE"] += R_STAGE_DVE * scols + F_DVE
            stage_eng = "DVE"
        plans.append((f, x, y, w, stage_eng))
    return plans, loads


def _build_nc():
    import concourse.tile as tile
    from concourse import bacc, mybir

    f32 = mybir.dt.float32
    f16 = mybir.dt.float16
    i8 = mybir.dt.int8
    u8 = mybir.dt.uint8

    plans, ledger = _plan(CHUNKS)
    kh16 = sum(p[0] for p in plans)
    kh8 = KH_PAD - kh16

    nc = bacc.Bacc("TRN2", target_bir_lowering=False, debug=False, num_devices=8)
    mw_d = nc.dram_tensor("mw", [N, KH_PAD], i8, kind="ExternalInput")
    mask8_d = nc.dram_tensor("mask8", [C * N, kh8], u8, kind="ExternalInput")
    mask16_d = nc.dram_tensor("mask16", [C * N, kh16], f16, kind="ExternalInput")
    mw16_d = nc.dram_tensor("mw16", [N, kh16], f16, kind="ExternalInput")
    w_ds = [
        nc.dram_tensor(f"w{ci}", [N, L], f16, kind="ExternalInput")
        for ci in range(C)
    ]
    # One output tensor per chunk keeps output DMAs independent (DRAM
    # deps are tracked per tensor). Layout [128, C * KBn * L]: partition
    # = k offset within block, free = (channel, k-block, l).
    pair_w = []
    for pi in range(0, len(CHUNKS), 2):
        w = C * (CHUNKS[pi] // KB) * L
        if pi + 1 < len(CHUNKS):
            w += C * (CHUNKS[pi + 1] // KB) * L
        pair_w.append(w)
    out_ds = [
        nc.dram_tensor(f"out{pi:02d}", [KB, w], f16, kind="ExternalOutput")
        for pi, w in enumerate(pair_w)
    ]

    mw_v = mw_d.ap().rearrange("(b p) k -> p b k", p=128)        # [128, 4, KHp]
    mask8_v = mask8_d.ap().rearrange("(b p) k -> p b k", p=128)  # [128, 8, kh8]
    mask16_v = mask16_d.ap().rearrange("(b p) k -> p b k", p=128)
    mw16_v = mw16_d.ap().rearrange("(b p) k -> p b k", p=128)
    w_vs = [w.ap().rearrange("(b p) l -> p b l", p=128) for w in w_ds]

    with tile.TileContext(nc) as tc:
        with (
            tc.tile_pool(name="const", bufs=1) as cpool,
            tc.tile_pool(name="mwp", bufs=IN_BUFS) as mwp,
            tc.tile_pool(name="maskp", bufs=IN_BUFS) as maskp,
            tc.tile_pool(name="cvp", bufs=4) as cvp,
            tc.tile_pool(name="stp", bufs=ST_BUFS) as stp,
            tc.tile_pool(name="obp", bufs=OB_BUFS) as obp,
            tc.tile_pool(name="pop", bufs=PSUM_BUFS, space="PSUM") as pop,
            tc.tile_pool(name="wpop", bufs=1, space="PSUM") as wpop,
        ):
            def copy_op(name, out, in_):
                if name == "ACT":
                    nc.scalar.copy(out, in_)
                elif name == "DVE":
                    nc.vector.tensor_copy(out, in_)
                else:
                    nc.gpsimd.tensor_copy(out, in_)

            def mul_op(name, out, in0, in1):
                if name == "DVE":
                    nc.vector.tensor_mul(out, in0, in1)
                else:
                    nc.gpsimd.tensor_mul(out, in0, in1)

            off8 = [0]
            off16 = [0]
            off16w = [0]
            wtps = []

            def load_weights():
                for ci in range(C):
                    wtp = cpool.tile([128, NB, L], f16, tag=f"wtp{ci}")
                    nc.scalar.dma_start(wtp[:], w_vs[ci])
                    wtps.append(wtp)
                # Optional: harmless matmuls on the weight tile keep the
                # PE busy through the fill (clock p-state ramp).
                if WARMUP_MM:
                    pfw = wpop.tile([NB * L, NB * L], f32, tag="warm")
                    for wi in range(WARMUP_MM):
                        nc.tensor.matmul(
                            pfw[:],
                            wtps[0][:].rearrange("p b l -> p (b l)"),
                            wtps[0][:].rearrange("p b l -> p (b l)"),
                            start=(wi == 0),
                            stop=(wi == WARMUP_MM - 1),
                        )

            def load_phase(ci, ck):
                """DMA in + conversions for chunk ci (one stage ahead)."""
                f, x, y, w, stage_eng = plans[ci]
                is16 = ci in F16_CHUNKS
                if not is16:
                    mwq = mwp.tile([128, NB, ck], i8, tag="mwq")
                    nc.sync.dma_start(mwq[:], mw_v[:, :, k0s[ci] : k0s[ci] + ck])
                mq16 = mq8 = None
                if f:
                    mq16 = maskp.tile([128, C, NB, f], f16, tag="maskq16",
                                      bufs=3)
                    s0 = off16[0]
                    off16[0] += f
                    nc.sync.dma_start(
                        mq16[:].rearrange("p c b k -> p (c b) k"),
                        mask16_v[:, :, s0 : s0 + f],
                    )
                if ck - f:
                    mq8 = maskp.tile([128, C, NB, ck - f], u8, tag="maskq8")
                    s0 = off8[0]
                    off8[0] += ck - f
                    nc.sync.dma_start(
                        mq8[:].rearrange("p c b k -> p (c b) k"),
                        mask8_v[:, :, s0 : s0 + ck - f],
                    )
                if ci == 0:
                    load_weights()

                mwf = mwfc = mkf = None
                if is16:
                    # pre-scaled fp16 mw ships directly: no conversion hop
                    mwf = cvp.tile([128, NB, ck], f16, tag="mwf")
                    nc.sync.dma_start(mwf[:], mw16_v[:, :, off16w[0] : off16w[0] + ck])
                    off16w[0] += ck
                elif y > x:
                    # only the conv band needs fp16 mw (mixed bands multiply
                    # the raw int8 mw directly)
                    mwfc = cvp.tile([128, NB, y - x], f16, tag="mwfc")
                    copy_op("ACT", mwfc[:], mwq[:, :, x:y])
                if y > x:
                    # merged both-channel mask conversion (u8 cols x..y
                    # live at mq8 cols x-f..y-f), split so the first
                    # conv-band mul can start at the halfway point
                    mkf = cvp.tile([128, C, NB, y - x], f16, tag="mkf",
                                   bufs=3)
                    nsp = _CFG.get("mkf_split", 1)
                    stp_ = -(-(y - x) // nsp)
                    for a in range(0, y - x, stp_):
                        b = min(y - x, a + stp_)
                        copy_op("ACT", mkf[:, :, :, a:b],
                                mq8[:, :, :, x - f + a : x - f + b])
                return (mq16, mq8, mkf, mwf, mwfc, mwq if not is16 else None)

            def bc(ap, cols):
                """mwf[:, :, cols] broadcast across the channel dim."""
                return ap.rearrange("p (c b) k -> p c b k", c=1).to_broadcast(
                    (128, C, NB, cols)
                )

            def mul_phase(ci, ck, state):
                """st multiplies for a chunk loaded one stage earlier; both
                channels per op via a broadcast mwf AP."""
                f, x, y, w, stage_eng = plans[ci]
                mq16, mq8, mkf, mwf, mwfc, mwq = state
                st = stp.tile([128, C, NB, ck], f16, tag="st")
                late = ci >= len(CHUNKS) - _CFG.get("split_tail", 1)
                if f > 0:
                    step = KB if late else f
                    for f0 in range(0, f, step):
                        f1 = min(f, f0 + step)
                        mul_op("DVE", st[:, :, :, f0:f1],
                               mq16[:, :, :, f0:f1],
                               bc(mwf[:, :, f0:f1], f1 - f0))
                if x > f:
                    step = KB if late else x - f
                    for a in range(f, x, step):
                        b = min(x, a + step)
                        mul_op("DVE", st[:, :, :, a:b],
                               mq8[:, :, :, a - f : b - f],
                               bc(mwq[:, :, a:b], b - a))
                if y > x:
                    step = KB if late else -(-(y - x) // _CFG.get("mkf_split", 1))
                    for a in range(x, y, step):
                        b = min(y, a + step)
                        mul_op("DVE", st[:, :, :, a:b],
                               mkf[:, :, :, a - x : b - x],
                               bc(mwfc[:, :, a - x : b - x], b - a))
                if y < ck:
                    pstep = (ck - y + 1) // 2 if (
                        ci >= len(CHUNKS) - _CFG.get("pool_split_tail", 0)
                        and ck - y >= 128
                    ) else ck - y
                    for a in range(y, ck, pstep):
                        b = min(ck, a + pstep)
                        mul_op("Pool", st[:, :, :, a:b],
                               mq8[:, :, :, a - f : b - f],
                               bc(mwq[:, :, a:b], b - a))
                return st

            out_q = []
            pair_ob = [None]
            rr = [0]

            def flush_outs(keep=0):
                while len(out_q) > keep:
                    dst, srcv = out_q.pop(0)
                    eng = (nc.sync, nc.scalar)[rr[0] % 2]
                    eng.dma_start(dst, srcv)
                    rr[0] += 1

            def back_phase(ci, ck, st):
                """GEMM with st stationary / W' moving: for each (channel,
                k-block) region accumulate the four n-subtiles into
                pf[:, c, kb, :]; then one stage copy and one queued
                output DMA for the whole chunk."""
                stage_eng = plans[ci][4]
                nkb = ck // KB
                pf = pop.tile([KB, C, nkb, L], f32, tag="po")
                for cc in range(C):
                    for kb in range(nkb):
                        for ni in range(NB):
                            nc.tensor.matmul(
                                pf[:, cc, kb, :],
                                st[:, cc, ni, kb * KB : (kb + 1) * KB],
                                wtps[cc][:, ni, :],
                                start=(ni == 0),
                                stop=(ni == NB - 1),
                            )
                pi = ci // 2
                w0 = C * nkb * L
                if ci % 2 == 0:
                    ob = obp.tile([KB, pair_w[pi]], f16, tag="ob")
                    pair_ob[0] = ob
                else:
                    ob = pair_ob[0]
                off = 0 if ci % 2 == 0 else pair_w[pi] - w0
                copy_op(
                    stage_eng,
                    ob[:, off : off + w0].rearrange(
                        "p (c b l) -> p c b l", c=C, b=nkb
                    ),
                    pf[:],
                )
                if ci % 2 == 1 or ci == len(CHUNKS) - 1:
                    out_q.append((out_ds[pi].ap(), ob[:]))

            k0s = []
            k0 = 0
            for ck in CHUNKS:
                k0s.append(k0)
                k0 += ck

            # three-stage software pipeline over chunks:
            #   iteration i: multiplies(i) | matmul/stage/out(i-1) | load(i+1)
            loaded = {0: load_phase(0, CHUNKS[0])}
            mulled = {}
            for i in range(len(CHUNKS) + 1):
                if i < len(CHUNKS):
                    mulled[i] = mul_phase(i, CHUNKS[i], loaded.pop(i))
                if i >= 1:
                    keep = OUT_DEFER - 1 if i < len(CHUNKS) - 1 else 0
                    flush_outs(keep=keep)
                    back_phase(i - 1, CHUNKS[i - 1], mulled.pop(i - 1))
                if i + 1 < len(CHUNKS):
                    loaded[i + 1] = load_phase(i + 1, CHUNKS[i + 1])
            flush_outs()

    nc.compile()
    return nc


def get_nc():
    if "nc" not in _CACHE:
        _CACHE["nc"] = _build_nc()
    return _CACHE["nc"]


def _col_split():
    plans, _ = _plan(CHUNKS)
    cols8, cols16 = [], []
    k0 = 0
    for (f, x, y, w, se), ck in zip(plans, CHUNKS):
        if f:
            cols16.append((k0, k0 + f))
        if ck - f:
            cols8.append((k0 + f, k0 + ck))
        k0 += ck
    return cols8, cols16


_COLS8, _COLS16 = _col_split()


def make_in_maps(mixture_w, est_mask, W):
    mixture_w = np.asarray(mixture_w, np.float32)
    est_mask = np.asarray(est_mask, np.float32)
    W = np.asarray(W, np.float32)
    in_maps = []
    for m in range(M):
        for kh in range(2):
            s0 = kh * KH
            mw = np.zeros((N, KH_PAD), np.float32)
            mw[:, :KH] = mixture_w[m, :, s0 : s0 + KH]
            s = np.abs(mw).max(axis=1) / 127.0            # [N]
            np.maximum(s, 1e-30, out=s)
            mw_q = np.rint(mw / s[:, None]).astype(np.int8)
            wp = (W.T * (s[:, None] * (2.0**SCALE_BITS / 255.0))).astype(
                np.float16
            )                                             # [N, L]
            if _COLS16:
                # pre-scaled fp16 mw for the fp16 chunks (no device conv)
                mwn = mw / s[:, None]
                mw16 = np.concatenate(
                    [mwn[:, a:b] for a, b in _COLS16], axis=1
                ).astype(np.float16)
            else:
                mw16 = np.zeros((N, 0), np.float16)
            mask = np.zeros((C * N, KH_PAD), np.float32)
            mask[:, :KH] = est_mask[m, :, :, s0 : s0 + KH].reshape(C * N, KH)
            m8 = np.concatenate(
                [np.rint(mask[:, a:b] * 255.0) for a, b in _COLS8], axis=1
            ).astype(np.uint8) if _COLS8 else np.zeros((C * N, 0), np.uint8)
            if _COLS16:
                # fp16 mask columns carry the 255 scale so W' stays shared
                m16 = np.concatenate(
                    [mask[:, a:b] * np.float32(255.0) for a, b in _COLS16],
                    axis=1,
                ).astype(np.float16)
            else:
                m16 = np.zeros((C * N, 0), np.float16)
            in_maps.append(
                {
                    "mw": np.ascontiguousarray(mw_q),
                    "mw16": np.ascontiguousarray(mw16),
                    "mask8": np.ascontiguousarray(m8),
                    "mask16": np.ascontiguousarray(m16),
                    "w0": np.ascontiguousarray(wp),
                    "w1": np.ascontiguousarray(wp),
                }
            )
    return in_maps


def stitch(results):
    """results: 8 per-core dicts of per-chunk frame arrays, (m, kh) order."""
    inv = np.float32(2.0**-SCALE_BITS)
    out = np.zeros((M, C, T), np.float32)
    for m in range(M):
        for kh in range(2):
            r = results[2 * m + kh]
            frs = []
            for ci, ck in enumerate(CHUNKS):
                pa = r[f"out{ci // 2:02d}"].astype(np.float32)
                nkb = ck // KB
                w0 = C * nkb * L
                a = pa[:, :w0] if ci % 2 == 0 else pa[:, -w0:]
                a = a.reshape(KB, C, nkb, L)
                # frames[c, l, k0+kb*KB+p] = a[p, c, kb, l]
                frs.append(a.transpose(1, 3, 2, 0).reshape(C, L, ck))
            fr = np.concatenate(frs, axis=2)[:, :, :KH] * inv  # [C, L, KH]
            half = np.zeros((C, HOP, QH), np.float32)
            for c in range(C):
                top = fr[c, 0:HOP]              # frames[r, j]
                bot = fr[c, HOP:L]              # frames[r+8, j]
                half[c, :, :KH] = top
                half[c, :, 1:] += bot
            # [C, HOP, QH] -> [C, TH] with t = 8q + r
            half_t = half.transpose(0, 2, 1).reshape(C, TH)
            if kh == 0:
                out[m, :, :TH] = half_t
            else:
                out[m, :, KH * HOP :] += half_t
    return out


def kernel(mixture_w, est_mask, W):
    from concourse.bass_utils import run_bass_kernel_spmd

    nc = get_nc()
    in_maps = make_in_maps(mixture_w, est_mask, W)
    res = run_bass_kernel_spmd(nc, in_maps, list(range(M * 2)))
    return stitch(list(res.results))


# revision 8
# speedup vs baseline: 1.0044x; 1.0044x over previous
"""ConvTasNet-style decoder kernel for Trainium2 (8 NeuronCores).

Computation (per batch m, channel c):
    s[n,k]    = mixture_w[n,k] * est_mask[c,n,k]          n=512, k=16000
    frames    = W @ s                                      [16, 16000]
    out[8q+r] = frames[r, q] + frames[r+8, q-1]            overlap-add, hop 8

Sharding: 8 cores = (m in 0..3) x (k-half in 0..1); each core handles both
channels of one (m, k-half); no cross-core communication. The final
overlap-add is a cheap O(T) host pass over the returned frame matrices.

The kernel is HBM-DMA-bound, so inputs ship 8-bit: mw -> int8 with a
per-row scale s[n], est_mask -> uint8 (x255). Both scales fold into the
stationary weights W'[n,l] = W[l,n] * s[n] * 2^12 / 255 (fp16; the 2^12
keeps W' out of fp16 subnormals and is divided back out on the host).
The k axis is zero-padded to KH_PAD (multiple of 128) on the host.

The mask multiply runs DIRECTLY on the 8-bit operands: the DVE and
GpSimd engines multiply u8 x i8 -> f16 at their 1-byte mixed rate, so no
mw upconversion pass exists for those bands. Work is cut into 512-column
chunks; per chunk the columns split into bands chosen by a solved
min-max ledger over engine rates + the DMA pipe:
  [0,x)  mixed u8*i8 on the DVE,
  [x,y)  ACT converts mask+mw to fp16, DVE multiplies at its 2x fp16
         rate (spends idle ACT capacity to relieve the DVE),
  [y,ck) mixed u8*i8 on GpSimd.
A few dedicated fp16 chunks (mask and pre-scaled mw ship as fp16, muls
on the DVE at the 2x rate) trade spare DMA bandwidth for vector-engine
relief; they sit whole so every mask DMA keeps a >=512B contiguous run
(sub-512B runs pay 2x in the DMA engines). The last two chunks are tiny
fp16 chunks so the post-DMA drain chain is short. Both channels of every
band go through ONE engine op via a channel-broadcast (stride-0) access
pattern on the mw operand.

The GEMM uses the product tile as the STATIONARY operand and W' as the
16-column moving operand: out[k-part, l] accumulates over the four
n-subtiles into a [128, C*KB*16] PSUM tile (one region per (channel,
k-block)). The PE row count per matmul is L=16, so the PE stays far off
the critical path at any clock p-state (on hardware the cost moves into
the weight-load path, the same total rows as the classic orientation),
and the full-128-partition PSUM layout makes the per-chunk stage copy
and output DMA 4x smaller than a 32-partition layout would be.

Scheduling: input DMAs issue on the SP queue with deep tile pools so the
DMA pipe stays saturated; each chunk's output DMA is deferred several
chunks (and alternates SP/ACT queues) so its semaphore wait is already
satisfied at issue and never head-of-line-blocks a sequencer; two
adjacent chunks share one staging tile and one output DMA (keeps the
out transfer's contiguous run at 512B and halves its issue overhead);
stage copies run on the DVE, whose matmul dependencies are produced by
its own earlier ops.
"""

import sys

sys.path.insert(0, "/opt/trn_rl_repo")

import numpy as np

M, C, N, K, L = 4, 2, 512, 16000, 16
HOP = L // 2            # 8
KH = K // 2             # 8000 frames per core
KH_PAD = 8064           # padded to a multiple of KB=128
QH = KH + 1             # 8001 output blocks per core per channel
TH = QH * HOP           # 64008 samples per core per channel
T = (K - 1) * HOP + L   # 128008 full output samples
KB = 128                # k-block: PSUM partitions per accumulation region
NB = N // 128           # 4 contraction subtiles
SCALE_BITS = 12         # W' = W * s[n] * 2^SCALE_BITS / 255

# ---- tunables ----
# u8 chunks carry mixed/conv/pool bands; dedicated fp16 chunks (DVE-only
# muls) sit between them so every mask DMA keeps a >=512B contiguous run
# (a sub-512B run pays 2x in the DMA model). Last chunk fp16 = short tail.
import os as _os
import json as _json
_CFG = _json.loads(_os.environ.get("K2_CFG", "{}"))
CHUNKS = _CFG.get("chunks", [512, 512, 512, 512, 640, 512, 512, 512, 512,
                             512, 512, 512, 512, 512, 512, 128, 128])
F16_CHUNKS = set(_CFG.get("f16_set", [i for i, ck in enumerate(CHUNKS) if ck in (256, 128)]))
CONV_KCOLS = _CFG.get("conv", 2900)
POOL_KCOLS = _CFG.get("pool", 2400)
WARMUP_MM = 0           # PE-clock warmup matmuls during the fill
IN_BUFS = _CFG.get("in_bufs", 10)
ST_BUFS = _CFG.get("st_bufs", 6)             # product-tile depth (consumed one chunk later)
OB_BUFS = 8
PSUM_BUFS = 8
STEP = 32               # planner column quantum
OUT_DEFER = _CFG.get("out_defer", 5)
POOL_CAP = _CFG.get("pool_cap", 224)
CONV_CAP = 384
_CACHE = {}

# Engine cost model (ns/col-of-128 + fixed ns/op) from TimelineSim's
# hw_specs: DVE 0.96GHz (2x when all operands are 2-byte), ACT/Pool
# 1.2GHz, Pool multiply efficiency 0.42, copy 0.60; fixed = SBUF/PSUM
# access latency charged per op.
R_DVE_F16 = 0.5208
R_DVE_MIX = 1.0417
R_DVE_CONV = 0.5208     # u8->f16 tensor_copy (2x_2p)
R_ACT = 0.8333          # any ACT copy/conv
R_POOL_MUL = 1.9841
R_STAGE_DVE = 1.0417    # psum f32 -> sbuf f16
R_STAGE_ACT = 0.8333
F_DVE = 115.0
F_ACT = 190.0
F_POOL = 100.0


def _dma_ns(total_bytes, elem_bytes):
    """TimelineSim DMA model: descriptors/16 * max(elem*mult/22.5, 7)."""
    if total_bytes == 0:
        return 0.0
    mult = 2.0 if elem_bytes < 512 else 1.0
    ns_per_desc = max(elem_bytes * mult / 22.5, 7.0)
    return (total_bytes / elem_bytes) / 16.0 * ns_per_desc


def _chunk_dma(ck, f):
    if f == ck:
        d = _dma_ns(512 * ck * 2, 2 * ck)           # mw fp16 (pre-scaled)
    else:
        d = _dma_ns(512 * ck, ck)                   # mw int8
    if f:
        d += _dma_ns(1024 * f * 2, 2 * f)           # mask fp16 part
    if ck - f:
        d += _dma_ns(1024 * (ck - f), ck - f)       # mask u8 part
    d += _dma_ns(2 * C * L * ck, C * L * ck // 32)  # frames out (paired)
    return d


def _plan(chunks):
    """Per-chunk (f, x, y, w, stage_eng) from solved global quotas.

    Global balance (LP over engine rates + DMA pipe, T ~ 40us): fp16 ~
    1280 k-cols (whole chunks), conv ~ 1440, pool ~ 2400, rest mixed;
    mw-conv all on ACT (w=0). The first two chunks skip conv so the
    GpSimd engine (the saturated pacer) starts as soon as the first u8
    mask lands; the final u8 chunk skips pool for a short tail.
    """
    assert sum(chunks) == KH_PAD, sum(chunks)
    n = len(chunks)
    u8s = [i for i in range(n) if i not in F16_CHUNKS]
    fq = {i: (chunks[i] if i in F16_CHUNKS else 0) for i in range(n)}
    cq = {i: 0 for i in range(n)}
    pq = {i: 0 for i in range(n)}
    # pool: lighter starter on chunk 0, uniform elsewhere (the tail u8
    # chunks keep pool bands so GpSimd absorbs tail work in parallel)
    if "pool_list" in _CFG:
        for i, v in zip(u8s, _CFG["pool_list"]):
            pq[i] = v
    else:
        pq[u8s[0]] = 128
        rem = POOL_KCOLS - 128
        for i in u8s[1:]:
            pq[i] = min(POOL_CAP, int(round(rem / (len(u8s) - 1) / 32)) * 32)
    if "conv_list" in _CFG:
        for i, v in zip(u8s, _CFG["conv_list"]):
            cq[i] = v
    else:
        for i in u8s[2:]:
            cq[i] = int(round(CONV_KCOLS / (len(u8s) - 2) / 32)) * 32
    loads = {"DVE": 0.0, "ACT": 0.0, "Pool": 0.0, "DMA": 1900.0}
    plans = []
    for ci, ck in enumerate(chunks):
        f = min(fq[ci], ck)
        pool = min(pq[ci], ck - f)
        conv = min(cq[ci], ck - f - pool)
        x = ck - pool - conv          # mixed band end
        y = ck - pool                 # conv band end
        # mw conv: fp16 chunks DMA a pre-scaled fp16 mw (no conv); ACT
        # converts everywhere else
        w = 0
        loads["DMA"] += _chunk_dma(ck, f)
        loads["DVE"] += C * NB * (R_DVE_F16 * (f + y - x) + R_DVE_MIX * (x - f))
        loads["DVE"] += F_DVE * ((f > 0) + (x > f) + (y > x))
        if y > x:
            loads["ACT"] += R_ACT * C * NB * (y - x) + F_ACT
        if y < ck:
            loads["Pool"] += R_POOL_MUL * C * NB * (ck - y) + F_POOL
        if y > x:
            # conv band also needs its mw columns upconverted on ACT
            loads["ACT"] += R_ACT * NB * (y - x) + F_ACT
        scols = C * L * ck // KB
        if not _CFG.get("stage_dve", True) and loads["ACT"] + R_STAGE_ACT * scols \
                + F_ACT < loads["DVE"] + (R_STAGE_DVE * scols + F_DVE):
            loads["ACT"] += R_STAGE_ACT * scols + F_ACT
            stage_eng = "ACT"
        else:
            loads["DVE"] += R_STAGE_DVE * scols + F_DVE
            stage_eng = "DVE"
        plans.append((f, x, y, w, stage_eng))
    return plans, loads


def _build_nc():
    import concourse.tile as tile
    from concourse import bacc, mybir

    f32 = mybir.dt.float32
    f16 = mybir.dt.float16
    i8 = mybir.dt.int8
    u8 = mybir.dt.uint8

    plans, ledger = _plan(CHUNKS)
    kh16 = sum(p[0] for p in plans)
    kh8 = KH_PAD - kh16

    nc = bacc.Bacc("TRN2", target_bir_lowering=False, debug=False, num_devices=8)
    mw_d = nc.dram_tensor("mw", [N, KH_PAD], i8, kind="ExternalInput")
    mask8_d = nc.dram_tensor("mask8", [C * N, kh8], u8, kind="ExternalInput")
    mask16_d = nc.dram_tensor("mask16", [C * N, kh16], f16, kind="ExternalInput")
    mw16_d = nc.dram_tensor("mw16", [N, kh16], f16, kind="ExternalInput")
    w_ds = [
        nc.dram_tensor(f"w{ci}", [128, NB * L], f16, kind="ExternalInput")
        for ci in range(C)
    ]
    # One output tensor per chunk keeps output DMAs independent (DRAM
    # deps are tracked per tensor). Layout [128, C * KBn * L]: partition
    # = k offset within block, free = (channel, k-block, l).
    pair_w = []
    for pi in range(0, len(CHUNKS), 2):
        w = C * (CHUNKS[pi] // KB) * L
        if pi + 1 < len(CHUNKS):
            w += C * (CHUNKS[pi + 1] // KB) * L
        pair_w.append(w)
    out_ds = [
        nc.dram_tensor(f"out{pi:02d}", [KB, w], f16, kind="ExternalOutput")
        for pi, w in enumerate(pair_w)
    ]

    mw_v = mw_d.ap().rearrange("(b p) k -> p b k", p=128)        # [128, 4, KHp]
    mask8_v = mask8_d.ap().rearrange("(b p) k -> p b k", p=128)  # [128, 8, kh8]
    mask16_v = mask16_d.ap().rearrange("(b p) k -> p b k", p=128)
    mw16_v = mw16_d.ap().rearrange("(b p) k -> p b k", p=128)
    w_vs = [w.ap().rearrange("p (b l) -> p b l", b=NB) for w in w_ds]

    with tile.TileContext(nc) as tc:
        with (
            tc.tile_pool(name="const", bufs=1) as cpool,
            tc.tile_pool(name="mwp", bufs=IN_BUFS) as mwp,
            tc.tile_pool(name="maskp", bufs=IN_BUFS) as maskp,
            tc.tile_pool(name="cvp", bufs=4) as cvp,
            tc.tile_pool(name="stp", bufs=ST_BUFS) as stp,
            tc.tile_pool(name="obp", bufs=OB_BUFS) as obp,
            tc.tile_pool(name="pop", bufs=PSUM_BUFS, space="PSUM") as pop,
            tc.tile_pool(name="wpop", bufs=1, space="PSUM") as wpop,
        ):
            def copy_op(name, out, in_):
                if name == "ACT":
                    nc.scalar.copy(out, in_)
                elif name == "DVE":
                    nc.vector.tensor_copy(out, in_)
                else:
                    nc.gpsimd.tensor_copy(out, in_)

            def mul_op(name, out, in0, in1):
                if name == "DVE":
                    nc.vector.tensor_mul(out, in0, in1)
                else:
                    nc.gpsimd.tensor_mul(out, in0, in1)

            off8 = [0]
            off16 = [0]
            off16w = [0]
            wtps = []

            def load_weights():
                for ci in range(C):
                    wtp = cpool.tile([128, NB, L], f16, tag=f"wtp{ci}")
                    nc.scalar.dma_start(wtp[:], w_vs[ci])
                    wtps.append(wtp)
                # Optional: harmless matmuls on the weight tile keep the
                # PE busy through the fill (clock p-state ramp).
                if WARMUP_MM:
                    pfw = wpop.tile([NB * L, NB * L], f32, tag="warm")
                    for wi in range(WARMUP_MM):
                        nc.tensor.matmul(
                            pfw[:],
                            wtps[0][:].rearrange("p b l -> p (b l)"),
                            wtps[0][:].rearrange("p b l -> p (b l)"),
                            start=(wi == 0),
                            stop=(wi == WARMUP_MM - 1),
                        )

            def load_phase(ci, ck):
                """DMA in + conversions for chunk ci (one stage ahead)."""
                f, x, y, w, stage_eng = plans[ci]
                is16 = ci in F16_CHUNKS
                if not is16:
                    mwq = mwp.tile([128, NB, ck], i8, tag="mwq")
                    nc.sync.dma_start(mwq[:], mw_v[:, :, k0s[ci] : k0s[ci] + ck])
                mq16 = mq8 = None
                if f:
                    mq16 = maskp.tile([128, C, NB, f], f16, tag="maskq16",
                                      bufs=3)
                    s0 = off16[0]
                    off16[0] += f
                    nc.sync.dma_start(
                        mq16[:].rearrange("p c b k -> p (c b) k"),
                        mask16_v[:, :, s0 : s0 + f],
                    )
                if ck - f:
                    mq8 = maskp.tile([128, C, NB, ck - f], u8, tag="maskq8")
                    s0 = off8[0]
                    off8[0] += ck - f
                    nc.sync.dma_start(
                        mq8[:].rearrange("p c b k -> p (c b) k"),
                        mask8_v[:, :, s0 : s0 + ck - f],
                    )
                if ci == _CFG.get("w_chunk", 0):
                    load_weights()

                mwf = mwfc = mkf = None
                if is16:
                    # pre-scaled fp16 mw ships directly: no conversion hop
                    mwf = cvp.tile([128, NB, ck], f16, tag="mwf")
                    nc.sync.dma_start(mwf[:], mw16_v[:, :, off16w[0] : off16w[0] + ck])
                    off16w[0] += ck
                elif y > x:
                    # only the conv band needs fp16 mw (mixed bands multiply
                    # the raw int8 mw directly)
                    mwfc = cvp.tile([128, NB, y - x], f16, tag="mwfc")
                    copy_op("ACT", mwfc[:], mwq[:, :, x:y])
                if y > x:
                    # merged both-channel mask conversion (u8 cols x..y
                    # live at mq8 cols x-f..y-f), split so the first
                    # conv-band mul can start at the halfway point
                    mkf = cvp.tile([128, C, NB, y - x], f16, tag="mkf",
                                   bufs=3)
                    nsp = _CFG.get("mkf_split", 1)
                    stp_ = -(-(y - x) // nsp)
                    for a in range(0, y - x, stp_):
                        b = min(y - x, a + stp_)
                        copy_op("ACT", mkf[:, :, :, a:b],
                                mq8[:, :, :, x - f + a : x - f + b])
                return (mq16, mq8, mkf, mwf, mwfc, mwq if not is16 else None)

            def bc(ap, cols):
                """mwf[:, :, cols] broadcast across the channel dim."""
                return ap.rearrange("p (c b) k -> p c b k", c=1).to_broadcast(
                    (128, C, NB, cols)
                )

            def mul_phase(ci, ck, state):
                """st multiplies for a chunk loaded one stage earlier; both
                channels per op via a broadcast mwf AP."""
                f, x, y, w, stage_eng = plans[ci]
                mq16, mq8, mkf, mwf, mwfc, mwq = state
                st = stp.tile([128, C, NB, ck], f16, tag="st")
                late = ci >= len(CHUNKS) - _CFG.get("split_tail", 1)
                if f > 0:
                    step = KB if late else f
                    for f0 in range(0, f, step):
                        f1 = min(f, f0 + step)
                        mul_op("DVE", st[:, :, :, f0:f1],
                               mq16[:, :, :, f0:f1],
                               bc(mwf[:, :, f0:f1], f1 - f0))
                if x > f:
                    step = KB if late else x - f
                    for a in range(f, x, step):
                        b = min(x, a + step)
                        mul_op("DVE", st[:, :, :, a:b],
                               mq8[:, :, :, a - f : b - f],
                               bc(mwq[:, :, a:b], b - a))
                if y > x:
                    step = KB if late else -(-(y - x) // _CFG.get("mkf_split", 1))
                    for a in range(x, y, step):
                        b = min(y, a + step)
                        mul_op("DVE", st[:, :, :, a:b],
                               mkf[:, :, :, a - x : b - x],
                               bc(mwfc[:, :, a - x : b - x], b - a))
                if y < ck:
                    pstep = (ck - y + 1) // 2 if (
                        ci >= len(CHUNKS) - _CFG.get("pool_split_tail", 0)
                        and ck - y >= 128
                    ) else ck - y
                    for a in range(y, ck, pstep):
                        b = min(ck, a + pstep)
                        mul_op("Pool", st[:, :, :, a:b],
                               mq8[:, :, :, a - f : b - f],
                               bc(mwq[:, :, a:b], b - a))
                return st

            out_q = []
            pair_ob = [None]
            rr = [0]

            def flush_outs(keep=0):
                while len(out_q) > keep:
                    dst, srcv = out_q.pop(0)
                    eng = (nc.sync, nc.scalar)[rr[0] % 2]
                    eng.dma_start(dst, srcv)
                    rr[0] += 1

            def back_phase(ci, ck, st):
                """GEMM with st stationary / W' moving: for each (channel,
                k-block) region accumulate the four n-subtiles into
                pf[:, c, kb, :]; then one stage copy and one queued
                output DMA for the whole chunk."""
                stage_eng = plans[ci][4]
                nkb = ck // KB
                pf = pop.tile([KB, C, nkb, L], f32, tag="po")
                for cc in range(C):
                    for kb in range(nkb):
                        for ni in range(NB):
                            nc.tensor.matmul(
                                pf[:, cc, kb, :],
                                st[:, cc, ni, kb * KB : (kb + 1) * KB],
                                wtps[cc][:, ni, :],
                                start=(ni == 0),
                                stop=(ni == NB - 1),
                            )
                pi = ci // 2
                w0 = C * nkb * L
                if ci % 2 == 0:
                    ob = obp.tile([KB, pair_w[pi]], f16, tag="ob")
                    pair_ob[0] = ob
                else:
                    ob = pair_ob[0]
                off = 0 if ci % 2 == 0 else pair_w[pi] - w0
                copy_op(
                    stage_eng,
                    ob[:, off : off + w0].rearrange(
                        "p (c b l) -> p c b l", c=C, b=nkb
                    ),
                    pf[:],
                )
                if ci % 2 == 1 or ci == len(CHUNKS) - 1:
                    out_q.append((out_ds[pi].ap(), ob[:]))

            k0s = []
            k0 = 0
            for ck in CHUNKS:
                k0s.append(k0)
                k0 += ck

            # three-stage software pipeline over chunks:
            #   iteration i: multiplies(i) | matmul/stage/out(i-1) | load(i+1)
            loaded = {0: load_phase(0, CHUNKS[0])}
            mulled = {}
            for i in range(len(CHUNKS) + 1):
                if i < len(CHUNKS):
                    mulled[i] = mul_phase(i, CHUNKS[i], loaded.pop(i))
                if i >= 1:
                    keep = OUT_DEFER - 1 if i < len(CHUNKS) - 1 else 0
                    flush_outs(keep=keep)
                    back_phase(i - 1, CHUNKS[i - 1], mulled.pop(i - 1))
                if i + 1 < len(CHUNKS):
                    loaded[i + 1] = load_phase(i + 1, CHUNKS[i + 1])
            flush_outs()

    nc.compile()
    return nc


def get_nc():
    if "nc" not in _CACHE:
        _CACHE["nc"] = _build_nc()
    return _CACHE["nc"]


def _col_split():
    plans, _ = _plan(CHUNKS)
    cols8, cols16 = [], []
    k0 = 0
    for (f, x, y, w, se), ck in zip(plans, CHUNKS):
        if f:
            cols16.append((k0, k0 + f))
        if ck - f:
            cols8.append((k0 + f, k0 + ck))
        k0 += ck
    return cols8, cols16


_COLS8, _COLS16 = _col_split()


def make_in_maps(mixture_w, est_mask, W):
    mixture_w = np.asarray(mixture_w, np.float32)
    est_mask = np.asarray(est_mask, np.float32)
    W = np.asarray(W, np.float32)
    in_maps = []
    for m in range(M):
        for kh in range(2):
            s0 = kh * KH
            mw = np.zeros((N, KH_PAD), np.float32)
            mw[:, :KH] = mixture_w[m, :, s0 : s0 + KH]
            s = np.abs(mw).max(axis=1) / 127.0            # [N]
            np.maximum(s, 1e-30, out=s)
            mw_q = np.rint(mw / s[:, None]).astype(np.int8)
            wp = (W.T * (s[:, None] * (2.0**SCALE_BITS / 255.0))).astype(
                np.float16
            )                                             # [N, L]
            wr = np.ascontiguousarray(
                wp.reshape(NB, 128, L).transpose(1, 0, 2).reshape(128, NB * L)
            )
            if _COLS16:
                # pre-scaled fp16 mw for the fp16 chunks (no device conv)
                mwn = mw / s[:, None]
                mw16 = np.concatenate(
                    [mwn[:, a:b] for a, b in _COLS16], axis=1
                ).astype(np.float16)
            else:
                mw16 = np.zeros((N, 0), np.float16)
            mask = np.zeros((C * N, KH_PAD), np.float32)
            mask[:, :KH] = est_mask[m, :, :, s0 : s0 + KH].reshape(C * N, KH)
            m8 = np.concatenate(
                [np.rint(mask[:, a:b] * 255.0) for a, b in _COLS8], axis=1
            ).astype(np.uint8) if _COLS8 else np.zeros((C * N, 0), np.uint8)
            if _COLS16:
                # fp16 mask columns carry the 255 scale so W' stays shared
                m16 = np.concatenate(
                    [mask[:, a:b] * np.float32(255.0) for a, b in _COLS16],
                    axis=1,
                ).astype(np.float16)
            else:
                m16 = np.zeros((C * N, 0), np.float16)
            in_maps.append(
                {
                    "mw": np.ascontiguousarray(mw_q),
                    "mw16": np.ascontiguousarray(mw16),
                    "mask8": np.ascontiguousarray(m8),
                    "mask16": np.ascontiguousarray(m16),
                    "w0": wr,
                    "w1": wr,
                }
            )
    return in_maps


def stitch(results):
    """results: 8 per-core dicts of per-chunk frame arrays, (m, kh) order."""
    inv = np.float32(2.0**-SCALE_BITS)
    out = np.zeros((M, C, T), np.float32)
    for m in range(M):
        for kh in range(2):
            r = results[2 * m + kh]
            frs = []
            for ci, ck in enumerate(CHUNKS):
                pa = r[f"out{ci // 2:02d}"].astype(np.float32)
                nkb = ck // KB
                w0 = C * nkb * L
                a = pa[:, :w0] if ci % 2 == 0 else pa[:, -w0:]
                a = a.reshape(KB, C, nkb, L)
                # frames[c, l, k0+kb*KB+p] = a[p, c, kb, l]
                frs.append(a.transpose(1, 3, 2, 0).reshape(C, L, ck))
            fr = np.concatenate(frs, axis=2)[:, :, :KH] * inv  # [C, L, KH]
            half = np.zeros((C, HOP, QH), np.float32)
            for c in range(C):
                top = fr[c, 0:HOP]              # frames[r, j]
                bot = fr[c, HOP:L]              # frames[r+8, j]
                half[c, :, :KH] = top
                half[c, :, 1:] += bot
            # [C, HOP, QH] -> [C, TH] with t = 8q + r
            half_t = half.transpose(0, 2, 1).reshape(C, TH)
            if kh == 0:
                out[m, :, :TH] = half_t
            else:
                out[m, :, KH * HOP :] += half_t
    return out


def kernel(mixture_w, est_mask, W):
    from concourse.bass_utils import run_bass_kernel_spmd

    nc = get_nc()
    in_maps = make_in_maps(mixture_w, est_mask, W)
    res = run_bass_kernel_spmd(nc, in_maps, list(range(M * 2)))
    return stitch(list(res.results))
